# revision 1
# baseline (speedup 1.0000x reference)
"""Causal single-head attention on 8 Trainium2 NeuronCores.

Math: out[b] = softmax(causal((x_b Wq^T)(x_b Wk^T)^T / 8)) @ (x_b Wv^T)

Strategy (pure batch data-parallelism, 512 batches/core):
  - Host precomputes A = (Wq^T Wk)/8 so scores = x A x^T (one projection saved).
  - Host transposes x to [B, C, T] so the contraction dim c lands on SBUF
    partitions with no on-device transpose; cast to bf16 (halves input DMA).
  - Two batches are packed per 128-partition tile (c-dim is 64).
  - Per batch on device (all matmuls bf16 operands, fp32 PSUM accumulate):
      gT   = blockdiag(A,A)^T @ xT_pair            (pair-packed, one matmul)
      sT   = x_b^T-stationary @ gT   -> scores^T[s,t] in PSUM
      mask : scores^T += -50 * tril_strict  via matmul(lhsT=LM, rhs=I)
      expT = ACT exp over 8 batches in one instruction (PSUM -> SBUF bf16)
      v    = x_b^T-stationary @ Wv^T                (natural [s,h] layout)
      U|Z  = expT-stationary @ [v | ones]           (U and Z in one matmul)
  - U, Z are DMAed out; the final out = U/Z division happens on host.
"""

import sys

sys.path.insert(0, "/opt/trn_rl_repo")

import numpy as np

B, T, C, H = 4096, 128, 64, 64
NCORES = 8
BPC = B // NCORES          # 512 batches per core
PAIRS = BPC // 2           # 256
GROUPS = PAIRS // 4        # 64 groups of 4 pairs (8 batches)
NEG = -50.0                # causal mask additive constant

_cache = {}


def _build(dtype_bf16):
    import concourse.bass as bass
    import concourse.bacc as bacc
    import concourse.mybir as mybir
    import concourse.tile as tile

    f32 = mybir.dt.float32
    bf16 = mybir.dt.bfloat16

    nc = bacc.Bacc("TRN2", target_bir_lowering=False, debug=False,
                   num_devices=NCORES)

    xt = nc.dram_tensor("xt", [GROUPS, 4, 128, 128], bf16, kind="ExternalInput")
    abd = nc.dram_tensor("abd", [128, 128], bf16, kind="ExternalInput")
    wvt2 = nc.dram_tensor("wvt2", [128, 128], bf16, kind="ExternalInput")
    lmask = nc.dram_tensor("lmask", [128, 128], bf16, kind="ExternalInput")
    ident = nc.dram_tensor("ident", [128, 512], bf16, kind="ExternalInput")
    uzout = nc.dram_tensor("uzout", [GROUPS, 128, 577], f32, kind="ExternalOutput")

    with tile.TileContext(nc) as tc:
        with (
            tc.tile_pool(name="const", bufs=1) as cpool,
            tc.tile_pool(name="sb", bufs=5) as sb,
            tc.tile_pool(name="psgv", bufs=2, space=bass.MemorySpace.PSUM) as psgv,
            tc.tile_pool(name="pss", bufs=2, space=bass.MemorySpace.PSUM) as pss,
        ):
            c_abd = cpool.tile([128, 128], bf16, tag="abd")
            c_wvt = cpool.tile([128, 128], bf16, tag="wvt")
            c_lm = cpool.tile([128, 128], bf16, tag="lm")
            c_id = cpool.tile([128, 512], bf16, tag="id")
            nc.sync.dma_start(c_abd[:], abd[:])
            nc.sync.dma_start(c_wvt[:], wvt2[:])
            nc.sync.dma_start(c_lm[:], lmask[:])
            nc.sync.dma_start(c_id[:], ident[:])

            # persistent v|ones tiles (double-buffered by hand); the ones
            # columns are written once and never touched again
            vo_a = cpool.tile([128, 520], bf16, tag="voa")
            vo_b = cpool.tile([128, 520], bf16, tag="vob")
            vo_bufs = [vo_a, vo_b]
            for vb in vo_bufs:
                nc.vector.memset(vb[:], 1.0)

            for g in range(GROUPS):
                sx = sb.tile([128, 512], bf16, tag="sx")
                nc.sync.dma_start(
                    sx[:].rearrange("p (k t) -> p k t", k=4),
                    xt[g].rearrange("k p t -> p k t"))

                pgv = psgv.tile([128, 1024], f32, tag="pgv")
                # gT for 4 pairs in one N=512 matmul: bank A
                nc.tensor.matmul(pgv[:, 0:512], c_abd[:], sx[:, 0:512],
                                 start=True, stop=True)
                # v via blockdiag(WvT,WvT): bank B
                for p in range(4):
                    nc.tensor.matmul(
                        pgv[:, 512 + 128 * p:512 + 128 * (p + 1)],
                        sx[:, 128 * p:128 * (p + 1)], c_wvt[:],
                        start=True, stop=True)

                sg = sb.tile([128, 512], bf16, tag="sg")
                nc.scalar.copy(sg[:], pgv[:, 0:512])

                vo = vo_bufs[g % 2]
                vo3 = vo[:].rearrange("p (b c) -> p b c", c=65)
                nc.vector.tensor_copy(
                    vo3[:, :, 0:64],
                    pgv[:, 512:1024].rearrange("p (b c) -> p b c", c=64))

                ps = pss.tile([128, 1024], f32, tag="ps")
                # scores^T[s, t]; row group hf=b%2 gets its own bank so
                # concurrent sub-array matmuls never share a PSUM bank
                def scol(b):
                    return 512 * (b % 2) + 128 * (b // 2)
                for b in range(8):
                    p, hf = b // 2, b % 2
                    xTb = sx[64 * hf:64 * (hf + 1), 128 * p:128 * (p + 1)]
                    gTb = sg[64 * hf:64 * (hf + 1), 128 * p:128 * (p + 1)]
                    nc.tensor.matmul(
                        ps[:, scol(b):scol(b) + 128], xTb, gTb,
                        start=(b < 2), stop=False,
                        skip_group_check=True)
                # causal mask accumulate: += -50 * 1[s > t], one MM per bank
                for bank in range(2):
                    nc.tensor.matmul(
                        ps[:, 512 * bank:512 * (bank + 1)], c_lm[:], c_id[:],
                        start=False, stop=True,
                        skip_group_check=True)

                se = sb.tile([128, 1024], bf16, tag="se")
                nc.scalar.activation(se[:], ps[:],
                                     mybir.ActivationFunctionType.Exp)

                # U|Z back into ps (scores are consumed): [t, 65] per batch
                for b in range(8):
                    col = 65 * b if b < 7 else 512
                    nc.tensor.matmul(
                        ps[:, col:col + 65],
                        se[:, scol(b):scol(b) + 128],
                        vo[:, 65 * b:65 * (b + 1)],
                        start=True, stop=True,
                        skip_group_check=True)

                so = sb.tile([128, 577], f32, tag="so")
                nc.vector.tensor_copy(so[:], ps[:, 0:577])
                nc.sync.dma_start(uzout[g], so[:])

    nc.compile()
    return nc


def _make_in_maps(x, Wq, Wk, Wv):
    import ml_dtypes

    x = np.asarray(x, dtype=np.float32)
    A = (np.asarray(Wq, np.float32).T @ np.asarray(Wk, np.float32)) / np.sqrt(H)
    abd = np.zeros((128, 128), np.float32)
    abd[0:64, 0:64] = A
    abd[64:128, 64:128] = A
    k_idx = np.arange(128)[:, None]
    s_idx = np.arange(128)[None, :]
    lm = np.where(s_idx > k_idx, np.float32(NEG), np.float32(0.0))
    ident = np.tile(np.eye(128, dtype=np.float32), (1, 4))

    wvT = np.asarray(Wv, np.float32).T
    wvt2 = np.zeros((128, 128), np.float32)
    wvt2[0:64, 0:64] = wvT
    wvt2[64:128, 64:128] = wvT

    bf = ml_dtypes.bfloat16
    consts = {
        "abd": abd.astype(bf),
        "wvt2": wvt2.astype(bf),
        "lmask": lm.astype(bf),
        "ident": ident.astype(bf),
    }

    # [B, T, C] -> per-core [GROUPS, 4(pairs), 2*C(stacked pair), T]
    xt_all = np.ascontiguousarray(x.transpose(0, 2, 1)).astype(bf)
    xt_all = xt_all.reshape(NCORES, GROUPS, 4, 128, 128)

    return [dict(consts, xt=np.ascontiguousarray(xt_all[i]))
            for i in range(NCORES)]


def kernel(x, Wq, Wk, Wv):
    from concourse.bass_utils import run_bass_kernel_spmd

    if "nc" not in _cache:
        _cache["nc"] = _build(True)
    nc = _cache["nc"]

    in_maps = _make_in_maps(x, Wq, Wk, Wv)
    res = run_bass_kernel_spmd(nc, in_maps, list(range(NCORES)))

    out = np.empty((B, T, H), np.float32)
    for i in range(NCORES):
        uzr = res.results[i]["uzout"]           # [GROUPS, 128, 577]
        uz = np.concatenate([uzr[:, :, 0:455], uzr[:, :, 512:577]], axis=2)
        uz = uz.reshape(GROUPS, 128, 8, 65)
        uz = np.moveaxis(uz, 2, 1).reshape(BPC, 128, 65)
        out[i * BPC:(i + 1) * BPC] = uz[:, :, 0:64] / uz[:, :, 64:65]
    return out

